# revision 36
# baseline (speedup 1.0000x reference)
"""Trainium2 Bass kernel for nn_Encoder (3 stacked strided 1-D convs).

The three convs (K=6, stride 2, valid) compose into a single 36-tap
stride-8 conv plus a constant bias:
    y[b, t] = sum_n w123[n] * x[b, 8t + n] + c

Strategy: pure data parallel over 8 NeuronCores (512 batch rows each).
Inside each core, batch rows sit on the 128 SBUF partitions (4 batch
tiles).  The host pre-splits x into its 8 polyphase planes
xph[b, p, m] = x[b, 8m + p], so every tap's moving operand is a
unit-stride slice.  The conv runs on the tensor engine as 36
accumulating "diagonal" matmuls per [128, 512] PSUM tile: stationary =
w123[k] * I_128 (bf16; contraction over the batch partition is a
per-row scale), moving = bf16 phase slice.  Weights are loaded once per
tap and reused across the 4 PSUM banks of a batch tile.
"""

import numpy as np

import concourse.bass as bass
import concourse.mybir as mybir
import concourse.tile as tile
from concourse import bacc
from concourse.bass_utils import run_bass_kernel_spmd

F32 = mybir.dt.float32
BF16 = mybir.dt.bfloat16

B, L = 4096, 16384
NCORES = 8
BC = B // NCORES          # 512 batch rows per core
P = 128                   # partitions
NBT = BC // P             # 4 batch tiles per core
K1 = 6
LOUT = 2044
KE, SE = 36, 8            # composite taps / stride
LP = L // SE              # 2048 phase length
FD = 512                  # psum free dim per matmul group
NJ = (LOUT + FD - 1) // FD  # 4 output column blocks (512,512,512,508)

_NC = None


def _build_program():
    # Bacc (not plain Bass): its compile() pass splits multi-sem waits into
    # event-semaphore instructions (TRN2 allows 1 wait per engine instr).
    nc = bacc.Bacc("TRN2", target_bir_lowering=False)
    x_d = nc.dram_tensor("x", [BC, SE, LP], F32, kind="ExternalInput")
    wd_d = nc.dram_tensor("wd", [P, KE * P], BF16, kind="ExternalInput")
    cv_d = nc.dram_tensor("cv", [P, 1], F32, kind="ExternalInput")
    wv_d = nc.dram_tensor("wv", [P, KE], F32, kind="ExternalInput")
    out_d = nc.dram_tensor("out", [BC, LOUT], F32, kind="ExternalOutput")

    # taps computed on ACT (multiply) + DVE (accumulate) instead of PE,
    # to pull tensor-engine time under the DMA roofline.  Early phases
    # (but not k=0/k=1, so PE starts immediately) let the ACT/DVE chain
    # run concurrently with PE instead of bunching at the tile tail.
    off_taps = {8, 16, 24, 32, 9, 17, 25, 33}

    with tile.TileContext(nc) as tc:
        with (
            tc.tile_pool(name="wpool", bufs=1) as wpool,
            tc.tile_pool(name="xpool", bufs=3) as xpool,
            tc.tile_pool(name="opool", bufs=2) as opool,
            tc.tile_pool(name="accpool", bufs=2 * NJ) as accpool,
            tc.tile_pool(name="tmppool", bufs=6) as tmppool,
            tc.tile_pool(name="psum", bufs=2 * NJ, space="PSUM") as ppool,
        ):
            wt = wpool.tile([P, KE * P], BF16)
            nc.sync.dma_start(wt[:], wd_d[:, :])
            cv = wpool.tile([P, 1], F32)
            nc.sync.dma_start(cv[:], cv_d[:, :])
            wv = wpool.tile([P, KE], F32)
            nc.sync.dma_start(wv[:], wv_d[:, :])

            # phase-major tap order: compute on phase p can start as soon
            # as that phase's DMA lands, hiding the x load latency.
            tap_order = [
                SE * jp + p
                for p in range(SE)
                for jp in range((KE - 1 - p) // SE + 1)
            ]
            assert sorted(tap_order) == list(range(KE))

            for bt in range(NBT):
                rows = slice(bt * P, (bt + 1) * P)
                xt = xpool.tile([P, SE, LP], BF16)
                prev_dma = None
                for p in range(SE):
                    # SWDGE DMA casts f32 HBM -> bf16 SBUF in flight
                    dma = nc.gpsimd.dma_start(xt[:, p, :], x_d[rows, p, :])
                    if bt == 0 and prev_dma is not None:
                        # bt0 prologue: all 8 phase DMAs otherwise drain
                        # concurrently and phase 0 lands ~whole-tile late;
                        # chaining them lets PE start after one phase.
                        from concourse.tile import add_dep_helper

                        add_dep_helper(
                            dma.ins, prev_dma.ins,
                            reason="bt0 phase DMA serial chain",
                        )
                    prev_dma = dma

                pss = [
                    ppool.tile([P, FD], F32, name=f"ps{j}", tag="ps")
                    for j in range(NJ)
                ]
                accs = [
                    accpool.tile([P, FD], F32, name=f"acc{j}", tag="acc")
                    for j in range(NJ)
                ]
                pe_taps = [k for k in tap_order if k not in off_taps]
                acc_started = [False] * NJ
                for ki, k in enumerate(tap_order):
                    p, jp = k % SE, k // SE
                    on_pe = k not in off_taps
                    lhsT = wt[:, k * P:(k + 1) * P]
                    for j in range(NJ):
                        nout = min(FD, LOUT - j * FD)
                        rhs = xt[:, p, j * FD + jp: j * FD + jp + nout]
                        if on_pe:
                            nc.tensor.matmul(
                                pss[j][:, :nout], lhsT, rhs,
                                start=(k == pe_taps[0]), stop=(k == pe_taps[-1]),
                            )
                        elif not acc_started[j]:
                            # acc = w_k * x + c  (bias folded in up front)
                            nc.scalar.activation(
                                accs[j][:, :nout], rhs,
                                mybir.ActivationFunctionType.Identity,
                                bias=cv[:, 0:1], scale=wv[:, k:k + 1],
                            )
                            acc_started[j] = True
                        else:
                            tmp = tmppool.tile([P, FD], BF16, name="tmp",
                                               tag="tmp")
                            nc.scalar.activation(
                                tmp[:, :nout], rhs,
                                mybir.ActivationFunctionType.Copy,
                                bias=0.0, scale=wv[:, k:k + 1],
                            )
                            nc.vector.tensor_add(
                                accs[j][:, :nout], accs[j][:, :nout],
                                tmp[:, :nout],
                            )

                osb = opool.tile([P, LOUT], F32)
                for j in range(NJ):
                    nout = min(FD, LOUT - j * FD)
                    # evacuate PSUM -> SBUF, folding in the ACT/DVE partials
                    nc.vector.tensor_add(
                        osb[:, j * FD: j * FD + nout], pss[j][:, :nout],
                        accs[j][:, :nout],
                    )
                nc.sync.dma_start(out_d[rows, :], osb[:])
    nc.finalize()
    return nc


def _composite(w1, b1, w2, b2, w3, b3):
    w1 = np.asarray(w1, np.float64).reshape(-1)
    w2 = np.asarray(w2, np.float64).reshape(-1)
    w3 = np.asarray(w3, np.float64).reshape(-1)
    b1 = float(np.asarray(b1).reshape(-1)[0])
    b2 = float(np.asarray(b2).reshape(-1)[0])
    b3 = float(np.asarray(b3).reshape(-1)[0])
    w12 = np.zeros(2 * (K1 - 1) + K1)
    for k1 in range(K1):
        for k2 in range(K1):
            w12[k1 + 2 * k2] += w1[k1] * w2[k2]
    w123 = np.zeros(KE)
    for m in range(len(w12)):
        for k3 in range(K1):
            w123[m + 4 * k3] += w12[m] * w3[k3]
    c = (b1 * w2.sum() + b2) * w3.sum() + b3
    return w123.astype(np.float32), np.float32(c)


def _run(x, w1, b1, w2, b2, w3, b3, trace=False):
    global _NC
    if _NC is None:
        _NC = _build_program()
    nc = _NC

    import ml_dtypes

    w123, c = _composite(w1, b1, w2, b2, w3, b3)
    wd = np.zeros((P, KE * P), np.float32)
    ar = np.arange(P)
    for k in range(KE):
        wd[ar, k * P + ar] = w123[k]
    wd = wd.astype(ml_dtypes.bfloat16)
    cv = np.full((P, 1), c, np.float32)
    wv = np.tile(w123.reshape(1, KE), (P, 1)).astype(np.float32)

    # polyphase split: xph[b, p, m] = x[b, 8m + p]
    xf = np.asarray(x, np.float32).reshape(B, L).reshape(B, LP, SE)
    in_maps = [
        {
            "x": np.ascontiguousarray(
                xf[i * BC:(i + 1) * BC].transpose(0, 2, 1)
            ),
            "wd": wd,
            "cv": cv,
            "wv": wv,
        }
        for i in range(NCORES)
    ]
    # A prior profiling session can leave the accelerator wedged; a cheap
    # sacrificial op absorbs the one-shot NRT_EXEC_UNIT_UNRECOVERABLE, and
    # the retry covers any remaining transient device error.
    try:
        import jax
        import jax.numpy as jnp

        jax.jit(lambda a: a + 1.0)(
            jnp.zeros((16, 16), jnp.float32)
        ).block_until_ready()
    except Exception:
        pass

    res = None
    for attempt in range(3):
        try:
            res = run_bass_kernel_spmd(
                nc, in_maps, core_ids=list(range(NCORES)), trace=trace
            )
            break
        except Exception:
            if attempt == 2:
                raise
            import time

            time.sleep(3.0)
    out = np.concatenate([res.results[i]["out"] for i in range(NCORES)], axis=0)
    return out.reshape(B, 1, LOUT), res


def kernel(x, w1, b1, w2, b2, w3, b3):
    out, _ = _run(x, w1, b1, w2, b2, w3, b3, trace=False)
    return out


# revision 39
# speedup vs baseline: 1.1075x; 1.1075x over previous
"""Trainium2 Bass kernel for nn_Encoder (3 stacked strided 1-D convs).

The three convs (K=6, stride 2, valid) compose into a single 36-tap
stride-8 conv plus a constant bias:
    y[b, t] = sum_n w123[n] * x[b, 8t + n] + c

Strategy: pure data parallel over 8 NeuronCores (512 batch rows each).
Inside each core, batch rows sit on the 128 SBUF partitions (4 batch
tiles).  The host pre-splits x into its 8 polyphase planes
xph[b, p, m] = x[b, 8m + p], so every tap's moving operand is a
unit-stride slice.  The conv runs on the tensor engine as 36
accumulating "diagonal" matmuls per [128, 512] PSUM tile: stationary =
w123[k] * I_128 (bf16; contraction over the batch partition is a
per-row scale), moving = bf16 phase slice.  Weights are loaded once per
tap and reused across the 4 PSUM banks of a batch tile.
"""

import numpy as np

import concourse.bass as bass
import concourse.mybir as mybir
import concourse.tile as tile
from concourse import bacc
from concourse.bass_utils import run_bass_kernel_spmd

F32 = mybir.dt.float32
BF16 = mybir.dt.bfloat16

B, L = 4096, 16384
NCORES = 8
BC = B // NCORES          # 512 batch rows per core
P = 128                   # partitions
NBT = BC // P             # 4 batch tiles per core
K1 = 6
LOUT = 2044
KE, SE = 36, 8            # composite taps / stride
LP = L // SE              # 2048 phase length
FD = 512                  # psum free dim per matmul group
NJ = (LOUT + FD - 1) // FD  # 4 output column blocks (512,512,512,508)

_NC = None


def _build_program():
    # Bacc (not plain Bass): its compile() pass splits multi-sem waits into
    # event-semaphore instructions (TRN2 allows 1 wait per engine instr).
    nc = bacc.Bacc("TRN2", target_bir_lowering=False)
    x_d = nc.dram_tensor("x", [BC, SE, LP], F32, kind="ExternalInput")
    wd_d = nc.dram_tensor("wd", [P, KE * P], BF16, kind="ExternalInput")
    cv_d = nc.dram_tensor("cv", [P, 1], F32, kind="ExternalInput")
    wv_d = nc.dram_tensor("wv", [P, KE], F32, kind="ExternalInput")
    out_d = nc.dram_tensor("out", [BC, LOUT], F32, kind="ExternalOutput")

    # taps computed on ACT (multiply) + DVE (accumulate) instead of PE,
    # to pull tensor-engine time under the DMA roofline.  Early phases
    # (but not k=0/k=1, so PE starts immediately) let the ACT/DVE chain
    # run concurrently with PE instead of bunching at the tile tail.
    off_taps = {8, 16, 24, 32, 9, 17, 25}

    with tile.TileContext(nc) as tc:
        with (
            tc.tile_pool(name="wpool", bufs=1) as wpool,
            tc.tile_pool(name="xpool", bufs=3) as xpool,
            tc.tile_pool(name="opool", bufs=2) as opool,
            tc.tile_pool(name="accpool", bufs=2 * NJ) as accpool,
            tc.tile_pool(name="tmppool", bufs=6) as tmppool,
            tc.tile_pool(name="psum", bufs=2 * NJ, space="PSUM") as ppool,
        ):
            wt = wpool.tile([P, KE * P], BF16)
            nc.sync.dma_start(wt[:], wd_d[:, :])
            cv = wpool.tile([P, 1], F32)
            nc.sync.dma_start(cv[:], cv_d[:, :])
            wv = wpool.tile([P, KE], F32)
            nc.sync.dma_start(wv[:], wv_d[:, :])

            # phase-major tap order: compute on phase p can start as soon
            # as that phase's DMA lands, hiding the x load latency.
            tap_order = [
                SE * jp + p
                for p in range(SE)
                for jp in range((KE - 1 - p) // SE + 1)
            ]
            assert sorted(tap_order) == list(range(KE))

            for bt in range(NBT):
                rows = slice(bt * P, (bt + 1) * P)
                xt = xpool.tile([P, SE, LP], BF16)
                for p in range(SE):
                    # SWDGE DMA casts f32 HBM -> bf16 SBUF in flight
                    nc.gpsimd.dma_start(xt[:, p, :], x_d[rows, p, :])

                pss = [
                    ppool.tile([P, FD], F32, name=f"ps{j}", tag="ps")
                    for j in range(NJ)
                ]
                accs = [
                    accpool.tile([P, FD], F32, name=f"acc{j}", tag="acc")
                    for j in range(NJ)
                ]
                pe_taps = [k for k in tap_order if k not in off_taps]
                acc_started = [False] * NJ
                for ki, k in enumerate(tap_order):
                    p, jp = k % SE, k // SE
                    on_pe = k not in off_taps
                    lhsT = wt[:, k * P:(k + 1) * P]
                    for j in range(NJ):
                        nout = min(FD, LOUT - j * FD)
                        rhs = xt[:, p, j * FD + jp: j * FD + jp + nout]
                        if on_pe:
                            nc.tensor.matmul(
                                pss[j][:, :nout], lhsT, rhs,
                                start=(k == pe_taps[0]), stop=(k == pe_taps[-1]),
                            )
                        elif not acc_started[j]:
                            # acc = w_k * x + c  (bias folded in up front)
                            nc.scalar.activation(
                                accs[j][:, :nout], rhs,
                                mybir.ActivationFunctionType.Identity,
                                bias=cv[:, 0:1], scale=wv[:, k:k + 1],
                            )
                            acc_started[j] = True
                        else:
                            tmp = tmppool.tile([P, FD], BF16, name="tmp",
                                               tag="tmp")
                            nc.scalar.activation(
                                tmp[:, :nout], rhs,
                                mybir.ActivationFunctionType.Copy,
                                bias=0.0, scale=wv[:, k:k + 1],
                            )
                            nc.vector.tensor_add(
                                accs[j][:, :nout], accs[j][:, :nout],
                                tmp[:, :nout],
                            )

                osb = opool.tile([P, LOUT], F32)
                for j in range(NJ):
                    nout = min(FD, LOUT - j * FD)
                    # evacuate PSUM -> SBUF, folding in the ACT/DVE partials
                    nc.vector.tensor_add(
                        osb[:, j * FD: j * FD + nout], pss[j][:, :nout],
                        accs[j][:, :nout],
                    )
                nc.sync.dma_start(out_d[rows, :], osb[:])
    nc.finalize()
    return nc


def _composite(w1, b1, w2, b2, w3, b3):
    w1 = np.asarray(w1, np.float64).reshape(-1)
    w2 = np.asarray(w2, np.float64).reshape(-1)
    w3 = np.asarray(w3, np.float64).reshape(-1)
    b1 = float(np.asarray(b1).reshape(-1)[0])
    b2 = float(np.asarray(b2).reshape(-1)[0])
    b3 = float(np.asarray(b3).reshape(-1)[0])
    w12 = np.zeros(2 * (K1 - 1) + K1)
    for k1 in range(K1):
        for k2 in range(K1):
            w12[k1 + 2 * k2] += w1[k1] * w2[k2]
    w123 = np.zeros(KE)
    for m in range(len(w12)):
        for k3 in range(K1):
            w123[m + 4 * k3] += w12[m] * w3[k3]
    c = (b1 * w2.sum() + b2) * w3.sum() + b3
    return w123.astype(np.float32), np.float32(c)


def _run(x, w1, b1, w2, b2, w3, b3, trace=False):
    global _NC
    if _NC is None:
        _NC = _build_program()
    nc = _NC

    import ml_dtypes

    w123, c = _composite(w1, b1, w2, b2, w3, b3)
    wd = np.zeros((P, KE * P), np.float32)
    ar = np.arange(P)
    for k in range(KE):
        wd[ar, k * P + ar] = w123[k]
    wd = wd.astype(ml_dtypes.bfloat16)
    cv = np.full((P, 1), c, np.float32)
    wv = np.tile(w123.reshape(1, KE), (P, 1)).astype(np.float32)

    # polyphase split: xph[b, p, m] = x[b, 8m + p]
    xf = np.asarray(x, np.float32).reshape(B, L).reshape(B, LP, SE)
    in_maps = [
        {
            "x": np.ascontiguousarray(
                xf[i * BC:(i + 1) * BC].transpose(0, 2, 1)
            ),
            "wd": wd,
            "cv": cv,
            "wv": wv,
        }
        for i in range(NCORES)
    ]
    # A prior profiling session can leave the accelerator wedged; a cheap
    # sacrificial op absorbs the one-shot NRT_EXEC_UNIT_UNRECOVERABLE, and
    # the retry covers any remaining transient device error.
    try:
        import jax
        import jax.numpy as jnp

        jax.jit(lambda a: a + 1.0)(
            jnp.zeros((16, 16), jnp.float32)
        ).block_until_ready()
    except Exception:
        pass

    res = None
    for attempt in range(3):
        try:
            res = run_bass_kernel_spmd(
                nc, in_maps, core_ids=list(range(NCORES)), trace=trace
            )
            break
        except Exception:
            if attempt == 2:
                raise
            import time

            time.sleep(3.0)
    out = np.concatenate([res.results[i]["out"] for i in range(NCORES)], axis=0)
    return out.reshape(B, 1, LOUT), res


_DRIVER = """
import sys
import numpy as np
sys.path.insert(0, sys.argv[1])
import kernel
d = np.load(sys.argv[2])
out, _ = kernel._run(**{k: d[k] for k in d.files}, trace=False)
np.save(sys.argv[3], out)
"""


def kernel(x, w1, b1, w2, b2, w3, b3):
    """Run in a subprocess: an accelerator-worker death (e.g. a stale
    profiling session wedging the first execute) is unrecoverable within
    a process, but a fresh process gets a fresh worker."""
    import os
    import subprocess
    import sys
    import tempfile

    with tempfile.TemporaryDirectory() as td:
        inp = os.path.join(td, "inp.npz")
        outp = os.path.join(td, "out.npy")
        drv = os.path.join(td, "drv.py")
        np.savez(inp, x=x, w1=w1, b1=b1, w2=w2, b2=b2, w3=w3, b3=b3)
        with open(drv, "w") as f:
            f.write(_DRIVER)
        kdir = os.path.dirname(os.path.abspath(__file__))
        last = None
        for _ in range(3):
            r = subprocess.run(
                [sys.executable, drv, kdir, inp, outp],
                capture_output=True, text=True, timeout=1800,
            )
            if r.returncode == 0 and os.path.exists(outp):
                return np.load(outp)
            last = r
        raise RuntimeError(
            f"kernel subprocess failed: {last.returncode}\n"
            f"{last.stdout[-2000:]}\n{last.stderr[-4000:]}"
        )


# revision 40
# speedup vs baseline: 1.1924x; 1.0767x over previous
"""Trainium2 Bass kernel for nn_Encoder (3 stacked strided 1-D convs).

The three convs (K=6, stride 2, valid) compose into a single 36-tap
stride-8 conv plus a constant bias:
    y[b, t] = sum_n w123[n] * x[b, 8t + n] + c

Strategy: pure data parallel over 8 NeuronCores (512 batch rows each).
Inside each core, batch rows sit on the 128 SBUF partitions (4 batch
tiles).  The host pre-splits x into its 8 polyphase planes
xph[b, p, m] = x[b, 8m + p], so every tap's moving operand is a
unit-stride slice.  The conv runs on the tensor engine as 36
accumulating "diagonal" matmuls per [128, 512] PSUM tile: stationary =
w123[k] * I_128 (bf16; contraction over the batch partition is a
per-row scale), moving = bf16 phase slice.  Weights are loaded once per
tap and reused across the 4 PSUM banks of a batch tile.
"""

import numpy as np

import concourse.bass as bass
import concourse.mybir as mybir
import concourse.tile as tile
from concourse import bacc
from concourse.bass_utils import run_bass_kernel_spmd

F32 = mybir.dt.float32
BF16 = mybir.dt.bfloat16

B, L = 4096, 16384
NCORES = 8
BC = B // NCORES          # 512 batch rows per core
P = 128                   # partitions
NBT = BC // P             # 4 batch tiles per core
K1 = 6
LOUT = 2044
KE, SE = 36, 8            # composite taps / stride
LP = L // SE              # 2048 phase length
FD = 512                  # psum free dim per matmul group
NJ = (LOUT + FD - 1) // FD  # 4 output column blocks (512,512,512,508)

_NC = None


def _build_program():
    # Bacc (not plain Bass): its compile() pass splits multi-sem waits into
    # event-semaphore instructions (TRN2 allows 1 wait per engine instr).
    nc = bacc.Bacc("TRN2", target_bir_lowering=False)
    x_d = nc.dram_tensor("x", [BC, SE, LP], F32, kind="ExternalInput")
    wd_d = nc.dram_tensor("wd", [P, KE * P], BF16, kind="ExternalInput")
    cv_d = nc.dram_tensor("cv", [P, 1], F32, kind="ExternalInput")
    wv_d = nc.dram_tensor("wv", [P, KE], F32, kind="ExternalInput")
    out_d = nc.dram_tensor("out", [BC, LOUT], F32, kind="ExternalOutput")

    # taps computed on ACT (multiply) + DVE (accumulate) instead of PE,
    # to pull tensor-engine time under the DMA roofline.  Early phases
    # (but not k=0/k=1, so PE starts immediately) let the ACT/DVE chain
    # run concurrently with PE instead of bunching at the tile tail.
    off_taps = {8, 16, 24, 32, 9, 17}

    with tile.TileContext(nc) as tc:
        with (
            tc.tile_pool(name="wpool", bufs=1) as wpool,
            tc.tile_pool(name="xpool", bufs=3) as xpool,
            tc.tile_pool(name="opool", bufs=2) as opool,
            tc.tile_pool(name="accpool", bufs=2 * NJ) as accpool,
            tc.tile_pool(name="tmppool", bufs=6) as tmppool,
            tc.tile_pool(name="psum", bufs=2 * NJ, space="PSUM") as ppool,
        ):
            wt = wpool.tile([P, KE * P], BF16)
            nc.sync.dma_start(wt[:], wd_d[:, :])
            cv = wpool.tile([P, 1], F32)
            nc.sync.dma_start(cv[:], cv_d[:, :])
            wv = wpool.tile([P, KE], F32)
            nc.sync.dma_start(wv[:], wv_d[:, :])

            # phase-major tap order: compute on phase p can start as soon
            # as that phase's DMA lands, hiding the x load latency.
            tap_order = [
                SE * jp + p
                for p in range(SE)
                for jp in range((KE - 1 - p) // SE + 1)
            ]
            assert sorted(tap_order) == list(range(KE))

            for bt in range(NBT):
                rows = slice(bt * P, (bt + 1) * P)
                xt = xpool.tile([P, SE, LP], BF16)
                for p in range(SE):
                    # SWDGE DMA casts f32 HBM -> bf16 SBUF in flight
                    nc.gpsimd.dma_start(xt[:, p, :], x_d[rows, p, :])

                pss = [
                    ppool.tile([P, FD], F32, name=f"ps{j}", tag="ps")
                    for j in range(NJ)
                ]
                accs = [
                    accpool.tile([P, FD], F32, name=f"acc{j}", tag="acc")
                    for j in range(NJ)
                ]
                pe_taps = [k for k in tap_order if k not in off_taps]
                acc_started = [False] * NJ
                for ki, k in enumerate(tap_order):
                    p, jp = k % SE, k // SE
                    on_pe = k not in off_taps
                    lhsT = wt[:, k * P:(k + 1) * P]
                    for j in range(NJ):
                        nout = min(FD, LOUT - j * FD)
                        rhs = xt[:, p, j * FD + jp: j * FD + jp + nout]
                        if on_pe:
                            nc.tensor.matmul(
                                pss[j][:, :nout], lhsT, rhs,
                                start=(k == pe_taps[0]), stop=(k == pe_taps[-1]),
                            )
                        elif not acc_started[j]:
                            # acc = w_k * x + c  (bias folded in up front)
                            nc.scalar.activation(
                                accs[j][:, :nout], rhs,
                                mybir.ActivationFunctionType.Identity,
                                bias=cv[:, 0:1], scale=wv[:, k:k + 1],
                            )
                            acc_started[j] = True
                        else:
                            tmp = tmppool.tile([P, FD], BF16, name="tmp",
                                               tag="tmp")
                            nc.scalar.activation(
                                tmp[:, :nout], rhs,
                                mybir.ActivationFunctionType.Copy,
                                bias=0.0, scale=wv[:, k:k + 1],
                            )
                            nc.vector.tensor_add(
                                accs[j][:, :nout], accs[j][:, :nout],
                                tmp[:, :nout],
                            )

                osb = opool.tile([P, LOUT], F32)
                for j in range(NJ):
                    nout = min(FD, LOUT - j * FD)
                    # evacuate PSUM -> SBUF, folding in the ACT/DVE partials
                    nc.vector.tensor_add(
                        osb[:, j * FD: j * FD + nout], pss[j][:, :nout],
                        accs[j][:, :nout],
                    )
                nc.sync.dma_start(out_d[rows, :], osb[:])
    nc.finalize()
    return nc


def _composite(w1, b1, w2, b2, w3, b3):
    w1 = np.asarray(w1, np.float64).reshape(-1)
    w2 = np.asarray(w2, np.float64).reshape(-1)
    w3 = np.asarray(w3, np.float64).reshape(-1)
    b1 = float(np.asarray(b1).reshape(-1)[0])
    b2 = float(np.asarray(b2).reshape(-1)[0])
    b3 = float(np.asarray(b3).reshape(-1)[0])
    w12 = np.zeros(2 * (K1 - 1) + K1)
    for k1 in range(K1):
        for k2 in range(K1):
            w12[k1 + 2 * k2] += w1[k1] * w2[k2]
    w123 = np.zeros(KE)
    for m in range(len(w12)):
        for k3 in range(K1):
            w123[m + 4 * k3] += w12[m] * w3[k3]
    c = (b1 * w2.sum() + b2) * w3.sum() + b3
    return w123.astype(np.float32), np.float32(c)


def _run(x, w1, b1, w2, b2, w3, b3, trace=False):
    global _NC
    if _NC is None:
        _NC = _build_program()
    nc = _NC

    import ml_dtypes

    w123, c = _composite(w1, b1, w2, b2, w3, b3)
    wd = np.zeros((P, KE * P), np.float32)
    ar = np.arange(P)
    for k in range(KE):
        wd[ar, k * P + ar] = w123[k]
    wd = wd.astype(ml_dtypes.bfloat16)
    cv = np.full((P, 1), c, np.float32)
    wv = np.tile(w123.reshape(1, KE), (P, 1)).astype(np.float32)

    # polyphase split: xph[b, p, m] = x[b, 8m + p]
    xf = np.asarray(x, np.float32).reshape(B, L).reshape(B, LP, SE)
    in_maps = [
        {
            "x": np.ascontiguousarray(
                xf[i * BC:(i + 1) * BC].transpose(0, 2, 1)
            ),
            "wd": wd,
            "cv": cv,
            "wv": wv,
        }
        for i in range(NCORES)
    ]
    # A prior profiling session can leave the accelerator wedged; a cheap
    # sacrificial op absorbs the one-shot NRT_EXEC_UNIT_UNRECOVERABLE, and
    # the retry covers any remaining transient device error.
    try:
        import jax
        import jax.numpy as jnp

        jax.jit(lambda a: a + 1.0)(
            jnp.zeros((16, 16), jnp.float32)
        ).block_until_ready()
    except Exception:
        pass

    res = None
    for attempt in range(3):
        try:
            res = run_bass_kernel_spmd(
                nc, in_maps, core_ids=list(range(NCORES)), trace=trace
            )
            break
        except Exception:
            if attempt == 2:
                raise
            import time

            time.sleep(3.0)
    out = np.concatenate([res.results[i]["out"] for i in range(NCORES)], axis=0)
    return out.reshape(B, 1, LOUT), res


_DRIVER = """
import sys
import numpy as np
sys.path.insert(0, sys.argv[1])
import kernel
d = np.load(sys.argv[2])
out, _ = kernel._run(**{k: d[k] for k in d.files}, trace=False)
np.save(sys.argv[3], out)
"""


def kernel(x, w1, b1, w2, b2, w3, b3):
    """Run in a subprocess: an accelerator-worker death (e.g. a stale
    profiling session wedging the first execute) is unrecoverable within
    a process, but a fresh process gets a fresh worker."""
    import os
    import subprocess
    import sys
    import tempfile

    with tempfile.TemporaryDirectory() as td:
        inp = os.path.join(td, "inp.npz")
        outp = os.path.join(td, "out.npy")
        drv = os.path.join(td, "drv.py")
        np.savez(inp, x=x, w1=w1, b1=b1, w2=w2, b2=b2, w3=w3, b3=b3)
        with open(drv, "w") as f:
            f.write(_DRIVER)
        kdir = os.path.dirname(os.path.abspath(__file__))
        last = None
        for _ in range(3):
            r = subprocess.run(
                [sys.executable, drv, kdir, inp, outp],
                capture_output=True, text=True, timeout=1800,
            )
            if r.returncode == 0 and os.path.exists(outp):
                return np.load(outp)
            last = r
        raise RuntimeError(
            f"kernel subprocess failed: {last.returncode}\n"
            f"{last.stdout[-2000:]}\n{last.stderr[-4000:]}"
        )


# revision 41
# speedup vs baseline: 1.2301x; 1.0316x over previous
"""Trainium2 Bass kernel for nn_Encoder (3 stacked strided 1-D convs).

The three convs (K=6, stride 2, valid) compose into a single 36-tap
stride-8 conv plus a constant bias:
    y[b, t] = sum_n w123[n] * x[b, 8t + n] + c

Strategy: pure data parallel over 8 NeuronCores (512 batch rows each).
Inside each core, batch rows sit on the 128 SBUF partitions (4 batch
tiles).  The host pre-splits x into its 8 polyphase planes
xph[b, p, m] = x[b, 8m + p], so every tap's moving operand is a
unit-stride slice.  The conv runs on the tensor engine as 36
accumulating "diagonal" matmuls per [128, 512] PSUM tile: stationary =
w123[k] * I_128 (bf16; contraction over the batch partition is a
per-row scale), moving = bf16 phase slice.  Weights are loaded once per
tap and reused across the 4 PSUM banks of a batch tile.
"""

import numpy as np

import concourse.bass as bass
import concourse.mybir as mybir
import concourse.tile as tile
from concourse import bacc
from concourse.bass_utils import run_bass_kernel_spmd

F32 = mybir.dt.float32
BF16 = mybir.dt.bfloat16

B, L = 4096, 16384
NCORES = 8
BC = B // NCORES          # 512 batch rows per core
P = 128                   # partitions
NBT = BC // P             # 4 batch tiles per core
K1 = 6
LOUT = 2044
KE, SE = 36, 8            # composite taps / stride
LP = L // SE              # 2048 phase length
FD = 512                  # psum free dim per matmul group
NJ = (LOUT + FD - 1) // FD  # 4 output column blocks (512,512,512,508)

_NC = None


def _build_program():
    # Bacc (not plain Bass): its compile() pass splits multi-sem waits into
    # event-semaphore instructions (TRN2 allows 1 wait per engine instr).
    nc = bacc.Bacc("TRN2", target_bir_lowering=False)
    x_d = nc.dram_tensor("x", [BC, SE, LP], F32, kind="ExternalInput")
    wd_d = nc.dram_tensor("wd", [P, KE * P], BF16, kind="ExternalInput")
    cv_d = nc.dram_tensor("cv", [P, 1], F32, kind="ExternalInput")
    wv_d = nc.dram_tensor("wv", [P, KE], F32, kind="ExternalInput")
    out_d = nc.dram_tensor("out", [BC, LOUT], F32, kind="ExternalOutput")

    # taps computed on ACT (multiply) + DVE (accumulate) instead of PE,
    # to pull tensor-engine time under the DMA roofline.  Early phases
    # (but not k=0/k=1, so PE starts immediately) let the ACT/DVE chain
    # run concurrently with PE instead of bunching at the tile tail.
    off_taps = {8, 16, 24, 32, 9, 17}

    with tile.TileContext(nc) as tc:
        with (
            tc.tile_pool(name="wpool", bufs=1) as wpool,
            tc.tile_pool(name="xpool", bufs=4) as xpool,
            tc.tile_pool(name="opool", bufs=2) as opool,
            tc.tile_pool(name="accpool", bufs=2 * NJ) as accpool,
            tc.tile_pool(name="tmppool", bufs=6) as tmppool,
            tc.tile_pool(name="psum", bufs=2 * NJ, space="PSUM") as ppool,
        ):
            wt = wpool.tile([P, KE * P], BF16)
            nc.sync.dma_start(wt[:], wd_d[:, :])
            cv = wpool.tile([P, 1], F32)
            nc.sync.dma_start(cv[:], cv_d[:, :])
            wv = wpool.tile([P, KE], F32)
            nc.sync.dma_start(wv[:], wv_d[:, :])

            # phase-major tap order: compute on phase p can start as soon
            # as that phase's DMA lands, hiding the x load latency.
            tap_order = [
                SE * jp + p
                for p in range(SE)
                for jp in range((KE - 1 - p) // SE + 1)
            ]
            assert sorted(tap_order) == list(range(KE))

            for bt in range(NBT):
                rows = slice(bt * P, (bt + 1) * P)
                xt = xpool.tile([P, SE, LP], BF16)
                for p in range(SE):
                    # SWDGE DMA casts f32 HBM -> bf16 SBUF in flight
                    nc.gpsimd.dma_start(xt[:, p, :], x_d[rows, p, :])

                pss = [
                    ppool.tile([P, FD], F32, name=f"ps{j}", tag="ps")
                    for j in range(NJ)
                ]
                accs = [
                    accpool.tile([P, FD], F32, name=f"acc{j}", tag="acc")
                    for j in range(NJ)
                ]
                pe_taps = [k for k in tap_order if k not in off_taps]
                acc_started = [False] * NJ
                for ki, k in enumerate(tap_order):
                    p, jp = k % SE, k // SE
                    on_pe = k not in off_taps
                    lhsT = wt[:, k * P:(k + 1) * P]
                    for j in range(NJ):
                        nout = min(FD, LOUT - j * FD)
                        rhs = xt[:, p, j * FD + jp: j * FD + jp + nout]
                        if on_pe:
                            nc.tensor.matmul(
                                pss[j][:, :nout], lhsT, rhs,
                                start=(k == pe_taps[0]), stop=(k == pe_taps[-1]),
                            )
                        elif not acc_started[j]:
                            # acc = w_k * x + c  (bias folded in up front)
                            nc.scalar.activation(
                                accs[j][:, :nout], rhs,
                                mybir.ActivationFunctionType.Identity,
                                bias=cv[:, 0:1], scale=wv[:, k:k + 1],
                            )
                            acc_started[j] = True
                        else:
                            tmp = tmppool.tile([P, FD], BF16, name="tmp",
                                               tag="tmp")
                            nc.scalar.activation(
                                tmp[:, :nout], rhs,
                                mybir.ActivationFunctionType.Copy,
                                bias=0.0, scale=wv[:, k:k + 1],
                            )
                            nc.vector.tensor_add(
                                accs[j][:, :nout], accs[j][:, :nout],
                                tmp[:, :nout],
                            )

                osb = opool.tile([P, LOUT], F32)
                for j in range(NJ):
                    nout = min(FD, LOUT - j * FD)
                    # evacuate PSUM -> SBUF, folding in the ACT/DVE partials
                    nc.vector.tensor_add(
                        osb[:, j * FD: j * FD + nout], pss[j][:, :nout],
                        accs[j][:, :nout],
                    )
                nc.sync.dma_start(out_d[rows, :], osb[:])
    nc.finalize()
    return nc


def _composite(w1, b1, w2, b2, w3, b3):
    w1 = np.asarray(w1, np.float64).reshape(-1)
    w2 = np.asarray(w2, np.float64).reshape(-1)
    w3 = np.asarray(w3, np.float64).reshape(-1)
    b1 = float(np.asarray(b1).reshape(-1)[0])
    b2 = float(np.asarray(b2).reshape(-1)[0])
    b3 = float(np.asarray(b3).reshape(-1)[0])
    w12 = np.zeros(2 * (K1 - 1) + K1)
    for k1 in range(K1):
        for k2 in range(K1):
            w12[k1 + 2 * k2] += w1[k1] * w2[k2]
    w123 = np.zeros(KE)
    for m in range(len(w12)):
        for k3 in range(K1):
            w123[m + 4 * k3] += w12[m] * w3[k3]
    c = (b1 * w2.sum() + b2) * w3.sum() + b3
    return w123.astype(np.float32), np.float32(c)


def _run(x, w1, b1, w2, b2, w3, b3, trace=False):
    global _NC
    if _NC is None:
        _NC = _build_program()
    nc = _NC

    import ml_dtypes

    w123, c = _composite(w1, b1, w2, b2, w3, b3)
    wd = np.zeros((P, KE * P), np.float32)
    ar = np.arange(P)
    for k in range(KE):
        wd[ar, k * P + ar] = w123[k]
    wd = wd.astype(ml_dtypes.bfloat16)
    cv = np.full((P, 1), c, np.float32)
    wv = np.tile(w123.reshape(1, KE), (P, 1)).astype(np.float32)

    # polyphase split: xph[b, p, m] = x[b, 8m + p]
    xf = np.asarray(x, np.float32).reshape(B, L).reshape(B, LP, SE)
    in_maps = [
        {
            "x": np.ascontiguousarray(
                xf[i * BC:(i + 1) * BC].transpose(0, 2, 1)
            ),
            "wd": wd,
            "cv": cv,
            "wv": wv,
        }
        for i in range(NCORES)
    ]
    # A prior profiling session can leave the accelerator wedged; a cheap
    # sacrificial op absorbs the one-shot NRT_EXEC_UNIT_UNRECOVERABLE, and
    # the retry covers any remaining transient device error.
    try:
        import jax
        import jax.numpy as jnp

        jax.jit(lambda a: a + 1.0)(
            jnp.zeros((16, 16), jnp.float32)
        ).block_until_ready()
    except Exception:
        pass

    res = None
    for attempt in range(3):
        try:
            res = run_bass_kernel_spmd(
                nc, in_maps, core_ids=list(range(NCORES)), trace=trace
            )
            break
        except Exception:
            if attempt == 2:
                raise
            import time

            time.sleep(3.0)
    out = np.concatenate([res.results[i]["out"] for i in range(NCORES)], axis=0)
    return out.reshape(B, 1, LOUT), res


_DRIVER = """
import sys
import numpy as np
sys.path.insert(0, sys.argv[1])
import kernel
d = np.load(sys.argv[2])
out, _ = kernel._run(**{k: d[k] for k in d.files}, trace=False)
np.save(sys.argv[3], out)
"""


def kernel(x, w1, b1, w2, b2, w3, b3):
    """Run in a subprocess: an accelerator-worker death (e.g. a stale
    profiling session wedging the first execute) is unrecoverable within
    a process, but a fresh process gets a fresh worker."""
    import os
    import subprocess
    import sys
    import tempfile

    with tempfile.TemporaryDirectory() as td:
        inp = os.path.join(td, "inp.npz")
        outp = os.path.join(td, "out.npy")
        drv = os.path.join(td, "drv.py")
        np.savez(inp, x=x, w1=w1, b1=b1, w2=w2, b2=b2, w3=w3, b3=b3)
        with open(drv, "w") as f:
            f.write(_DRIVER)
        kdir = os.path.dirname(os.path.abspath(__file__))
        last = None
        for _ in range(3):
            r = subprocess.run(
                [sys.executable, drv, kdir, inp, outp],
                capture_output=True, text=True, timeout=1800,
            )
            if r.returncode == 0 and os.path.exists(outp):
                return np.load(outp)
            last = r
        raise RuntimeError(
            f"kernel subprocess failed: {last.returncode}\n"
            f"{last.stdout[-2000:]}\n{last.stderr[-4000:]}"
        )


# revision 43
# speedup vs baseline: 1.3122x; 1.0668x over previous
"""Trainium2 Bass kernel for nn_Encoder (3 stacked strided 1-D convs).

The three convs (K=6, stride 2, valid) compose into a single 36-tap
stride-8 conv plus a constant bias:
    y[b, t] = sum_n w123[n] * x[b, 8t + n] + c

Strategy: pure data parallel over 8 NeuronCores (512 batch rows each).
Inside each core, batch rows sit on the 128 SBUF partitions (4 batch
tiles).  The host pre-splits x into its 8 polyphase planes
xph[b, p, m] = x[b, 8m + p], so every tap's moving operand is a
unit-stride slice.  The conv runs on the tensor engine as 36
accumulating "diagonal" matmuls per [128, 512] PSUM tile: stationary =
w123[k] * I_128 (bf16; contraction over the batch partition is a
per-row scale), moving = bf16 phase slice.  Weights are loaded once per
tap and reused across the 4 PSUM banks of a batch tile.
"""

import numpy as np

import concourse.bass as bass
import concourse.mybir as mybir
import concourse.tile as tile
from concourse import bacc
from concourse.bass_utils import run_bass_kernel_spmd

F32 = mybir.dt.float32
BF16 = mybir.dt.bfloat16

B, L = 4096, 16384
NCORES = 8
BC = B // NCORES          # 512 batch rows per core
P = 128                   # partitions
NBT = BC // P             # 4 batch tiles per core
K1 = 6
LOUT = 2044
KE, SE = 36, 8            # composite taps / stride
LP = L // SE              # 2048 phase length
FD = 512                  # psum free dim per matmul group
NJ = (LOUT + FD - 1) // FD  # 4 output column blocks (512,512,512,508)

_NC = None


def _build_program():
    # Bacc (not plain Bass): its compile() pass splits multi-sem waits into
    # event-semaphore instructions (TRN2 allows 1 wait per engine instr).
    nc = bacc.Bacc("TRN2", target_bir_lowering=False)
    x_d = nc.dram_tensor("x", [BC, SE, LP], F32, kind="ExternalInput")
    wd_d = nc.dram_tensor("wd", [P, KE * P], BF16, kind="ExternalInput")
    cv_d = nc.dram_tensor("cv", [P, 1], F32, kind="ExternalInput")
    wv_d = nc.dram_tensor("wv", [P, KE], F32, kind="ExternalInput")
    out_d = nc.dram_tensor("out", [BC, LOUT], F32, kind="ExternalOutput")

    # taps computed on ACT (multiply) + DVE (accumulate) instead of PE,
    # to pull tensor-engine time under the DMA roofline.  Early phases
    # (but not k=0/k=1, so PE starts immediately) let the ACT/DVE chain
    # run concurrently with PE instead of bunching at the tile tail.
    off_taps = {8, 16, 24, 32, 9, 17, 25, 33}

    with tile.TileContext(nc) as tc:
        with (
            tc.tile_pool(name="wpool", bufs=1) as wpool,
            tc.tile_pool(name="xpool", bufs=4) as xpool,
            tc.tile_pool(name="opool", bufs=2) as opool,
            tc.tile_pool(name="accpool", bufs=2 * NJ) as accpool,
            tc.tile_pool(name="tmppool", bufs=6) as tmppool,
            tc.tile_pool(name="psum", bufs=2 * NJ, space="PSUM") as ppool,
        ):
            wt = wpool.tile([P, KE * P], BF16)
            nc.sync.dma_start(wt[:], wd_d[:, :])
            cv = wpool.tile([P, 1], F32)
            nc.sync.dma_start(cv[:], cv_d[:, :])
            wv = wpool.tile([P, KE], F32)
            nc.sync.dma_start(wv[:], wv_d[:, :])

            # phase-major tap order: compute on phase p can start as soon
            # as that phase's DMA lands, hiding the x load latency.
            tap_order = [
                SE * jp + p
                for p in range(SE)
                for jp in range((KE - 1 - p) // SE + 1)
            ]
            assert sorted(tap_order) == list(range(KE))

            for bt in range(NBT):
                rows = slice(bt * P, (bt + 1) * P)
                xt = xpool.tile([P, SE, LP], BF16)
                for p in range(SE):
                    # SWDGE DMA casts f32 HBM -> bf16 SBUF in flight
                    nc.gpsimd.dma_start(xt[:, p, :], x_d[rows, p, :])

                pss = [
                    ppool.tile([P, FD], F32, name=f"ps{j}", tag="ps")
                    for j in range(NJ)
                ]
                # offload accumulators cover PAIRS of output blocks
                # (FD*2 wide) so the slow per-op ACT/DVE overheads halve
                accs = [
                    accpool.tile([P, 2 * FD], F32, name=f"acc{q}", tag="acc")
                    for q in range(NJ // 2)
                ]
                pw = [min(2 * FD, LOUT - q * 2 * FD) for q in range(NJ // 2)]
                pe_taps = [k for k in tap_order if k not in off_taps]
                acc_started = [False] * (NJ // 2)
                for ki, k in enumerate(tap_order):
                    p, jp = k % SE, k // SE
                    lhsT = wt[:, k * P:(k + 1) * P]
                    if k not in off_taps:
                        for j in range(NJ):
                            nout = min(FD, LOUT - j * FD)
                            rhs = xt[:, p, j * FD + jp: j * FD + jp + nout]
                            nc.tensor.matmul(
                                pss[j][:, :nout], lhsT, rhs,
                                start=(k == pe_taps[0]), stop=(k == pe_taps[-1]),
                            )
                        continue
                    for q in range(NJ // 2):
                        w = pw[q]
                        rhs = xt[:, p, q * 2 * FD + jp: q * 2 * FD + jp + w]
                        if not acc_started[q]:
                            # acc = w_k * x + c  (bias folded in up front)
                            nc.scalar.activation(
                                accs[q][:, :w], rhs,
                                mybir.ActivationFunctionType.Identity,
                                bias=cv[:, 0:1], scale=wv[:, k:k + 1],
                            )
                        else:
                            tmp = tmppool.tile([P, 2 * FD], BF16, name="tmp",
                                               tag="tmp")
                            nc.scalar.activation(
                                tmp[:, :w], rhs,
                                mybir.ActivationFunctionType.Copy,
                                bias=0.0, scale=wv[:, k:k + 1],
                            )
                            nc.vector.tensor_add(
                                accs[q][:, :w], accs[q][:, :w], tmp[:, :w],
                            )
                    acc_started = [True] * (NJ // 2)

                osb = opool.tile([P, LOUT], F32)
                for j in range(NJ):
                    nout = min(FD, LOUT - j * FD)
                    q, half = j // 2, (j % 2) * FD
                    # evacuate PSUM -> SBUF, folding in the ACT/DVE partials
                    nc.vector.tensor_add(
                        osb[:, j * FD: j * FD + nout], pss[j][:, :nout],
                        accs[q][:, half: half + nout],
                    )
                nc.sync.dma_start(out_d[rows, :], osb[:])
    nc.finalize()
    return nc


def _composite(w1, b1, w2, b2, w3, b3):
    w1 = np.asarray(w1, np.float64).reshape(-1)
    w2 = np.asarray(w2, np.float64).reshape(-1)
    w3 = np.asarray(w3, np.float64).reshape(-1)
    b1 = float(np.asarray(b1).reshape(-1)[0])
    b2 = float(np.asarray(b2).reshape(-1)[0])
    b3 = float(np.asarray(b3).reshape(-1)[0])
    w12 = np.zeros(2 * (K1 - 1) + K1)
    for k1 in range(K1):
        for k2 in range(K1):
            w12[k1 + 2 * k2] += w1[k1] * w2[k2]
    w123 = np.zeros(KE)
    for m in range(len(w12)):
        for k3 in range(K1):
            w123[m + 4 * k3] += w12[m] * w3[k3]
    c = (b1 * w2.sum() + b2) * w3.sum() + b3
    return w123.astype(np.float32), np.float32(c)


def _run(x, w1, b1, w2, b2, w3, b3, trace=False):
    global _NC
    if _NC is None:
        _NC = _build_program()
    nc = _NC

    import ml_dtypes

    w123, c = _composite(w1, b1, w2, b2, w3, b3)
    wd = np.zeros((P, KE * P), np.float32)
    ar = np.arange(P)
    for k in range(KE):
        wd[ar, k * P + ar] = w123[k]
    wd = wd.astype(ml_dtypes.bfloat16)
    cv = np.full((P, 1), c, np.float32)
    wv = np.tile(w123.reshape(1, KE), (P, 1)).astype(np.float32)

    # polyphase split: xph[b, p, m] = x[b, 8m + p]
    xf = np.asarray(x, np.float32).reshape(B, L).reshape(B, LP, SE)
    in_maps = [
        {
            "x": np.ascontiguousarray(
                xf[i * BC:(i + 1) * BC].transpose(0, 2, 1)
            ),
            "wd": wd,
            "cv": cv,
            "wv": wv,
        }
        for i in range(NCORES)
    ]
    # A prior profiling session can leave the accelerator wedged; a cheap
    # sacrificial op absorbs the one-shot NRT_EXEC_UNIT_UNRECOVERABLE, and
    # the retry covers any remaining transient device error.
    try:
        import jax
        import jax.numpy as jnp

        jax.jit(lambda a: a + 1.0)(
            jnp.zeros((16, 16), jnp.float32)
        ).block_until_ready()
    except Exception:
        pass

    res = None
    for attempt in range(3):
        try:
            res = run_bass_kernel_spmd(
                nc, in_maps, core_ids=list(range(NCORES)), trace=trace
            )
            break
        except Exception:
            if attempt == 2:
                raise
            import time

            time.sleep(3.0)
    out = np.concatenate([res.results[i]["out"] for i in range(NCORES)], axis=0)
    return out.reshape(B, 1, LOUT), res


_DRIVER = """
import sys
import numpy as np
sys.path.insert(0, sys.argv[1])
import kernel
d = np.load(sys.argv[2])
out, _ = kernel._run(**{k: d[k] for k in d.files}, trace=False)
np.save(sys.argv[3], out)
"""


def kernel(x, w1, b1, w2, b2, w3, b3):
    """Run in a subprocess: an accelerator-worker death (e.g. a stale
    profiling session wedging the first execute) is unrecoverable within
    a process, but a fresh process gets a fresh worker."""
    import os
    import subprocess
    import sys
    import tempfile

    with tempfile.TemporaryDirectory() as td:
        inp = os.path.join(td, "inp.npz")
        outp = os.path.join(td, "out.npy")
        drv = os.path.join(td, "drv.py")
        np.savez(inp, x=x, w1=w1, b1=b1, w2=w2, b2=b2, w3=w3, b3=b3)
        with open(drv, "w") as f:
            f.write(_DRIVER)
        kdir = os.path.dirname(os.path.abspath(__file__))
        last = None
        for _ in range(3):
            r = subprocess.run(
                [sys.executable, drv, kdir, inp, outp],
                capture_output=True, text=True, timeout=1800,
            )
            if r.returncode == 0 and os.path.exists(outp):
                return np.load(outp)
            last = r
        raise RuntimeError(
            f"kernel subprocess failed: {last.returncode}\n"
            f"{last.stdout[-2000:]}\n{last.stderr[-4000:]}"
        )
